# revision 1
# baseline (speedup 1.0000x reference)
"""Trainium2 Bass kernel for CircuitThermodynamics.

Strategy (pure data-parallel over batch, 8 cores x 512 rows):
  - ce @ W1 is factored through the 4-entry embedding table on the host:
        A1[t*256+g, f] = sum_d emb[t, d] * W1[g*32+d, f]
    so the device matmul contracts over a 1024-dim one-hot instead of the
    8192-dim materialized circuit embedding (8x fewer FLOPs, no gather).
    Four extra columns of A1 produce the per-row gate-type counts.
  - connections ([512, 65536] f32 per core, 128 MiB) is the DMA-bound bulk;
    it streams through SBUF in [128, 8192] tiles and is free-dim reduced by
    DVE (tensor_scalar + accum_out) and ACT (Copy + accum_out) in parallel,
    fully hidden under the DMA stream. The final row-chunk uses narrower
    tiles so the tail reduce is short.
  - conn DMAs own the sync-engine HWDGE ring; constant loads go through the
    scalar-engine ring so both start at t=0.
  - num_conn partials are flipped to free-major via a tiny PE transpose per
    row-chunk (no DRAM round-trip); heads/entropy epilogues run on
    [1, 512] / [4, 512] vectors in the transposed layout.
"""

import math
import sys

import numpy as np

for _p in ("/opt/trn_rl_repo", "/root/.axon_site/_ro/trn_rl_repo"):
    if _p not in sys.path:
        sys.path.append(_p)

import concourse.bacc as bacc
import concourse.mybir as mybir
from concourse.bass_utils import run_bass_kernel_spmd
from concourse.tile import TileContext

f32 = mybir.dt.float32
AF = mybir.ActivationFunctionType
ALU = mybir.AluOpType
AX = mybir.AxisListType

B, G, D = 4096, 256, 32
CE = G * D               # 8192
N_TYPES = 4
N_IO = 12                # 8 inputs + 4 outputs
N_CORES = 8
R = B // N_CORES         # 512 rows per core
CONN_F = G * G           # 65536
K1 = N_TYPES * G         # 1024 one-hot dim
F1 = 128 * 3 + 256       # 640 fused first-layer width
FT = F1 + N_TYPES        # +4 count columns
LN2_INV = 1.4426950408889634

# conn tile plan per row-chunk: (free_size, engine) — 'D' DVE, 'A' ACT.
# ~5:3 DVE:ACT balances 123 vs 154 Gelem/s engine rates under the DMA roof.
CONN_PLAN = [(8192, e) for e in "DDDDDAAA"]
# last chunk: narrow tail tiles, final two on different engines so their
# reduces overlap; shortens the post-stream critical path.
CONN_PLAN_LAST = [(8192, e) for e in "DDDAAA"] + [
    (4096, "D"), (4096, "A"), (4096, "D"), (4096, "A"),
]


def build_program(rows=R):
    """Build the single-core Bass/Tile program for `rows` batch rows."""
    rc = rows // 128
    nc = bacc.Bacc()

    conn_d = nc.dram_tensor("conn", [rows, CONN_F], f32, kind="ExternalInput")
    gtt_d = nc.dram_tensor("gtt", [G, rows], f32, kind="ExternalInput")
    iot_d = nc.dram_tensor("iot", [N_IO, rows], f32, kind="ExternalInput")
    a1_d = nc.dram_tensor("a1", [K1, FT], f32, kind="ExternalInput")
    b1_d = nc.dram_tensor("b1", [F1], f32, kind="ExternalInput")
    w1io_d = nc.dram_tensor("w1io", [N_IO, 256], f32, kind="ExternalInput")
    cw2_d = nc.dram_tensor("cw2", [256, 128], f32, kind="ExternalInput")
    cw3_d = nc.dram_tensor("cw3", [128, 1], f32, kind="ExternalInput")
    cb2_d = nc.dram_tensor("cb2", [128], f32, kind="ExternalInput")
    w2h_d = nc.dram_tensor("w2h", [128, 3], f32, kind="ExternalInput")
    scal_d = nc.dram_tensor("scal", [8], f32, kind="ExternalInput")
    ident_d = nc.dram_tensor("ident", [128, 128], f32, kind="ExternalInput")

    out_names = ["energy", "entropy", "stability", "correctness", "delay"]
    outs_d = {
        n: nc.dram_tensor(n, [rows], f32, kind="ExternalOutput") for n in out_names
    }

    with TileContext(nc) as tc:
        with (
            tc.tile_pool(name="consts", bufs=1) as cp,
            tc.tile_pool(name="conn", bufs=3) as connp,
            tc.tile_pool(name="vecs", bufs=8) as vp,
            tc.tile_pool(name="h1psum", bufs=2, space="PSUM") as php,
            tc.tile_pool(name="vpsum", bufs=3, space="PSUM") as pvp,
        ):
            def vtile(name, parts=1):
                return vp.tile([parts, rows], f32, name=name, tag="vec")

            # ---- constant loads (scalar-engine HWDGE ring) ----
            a1_t = []
            for k in range(K1 // 128):
                a1k = cp.tile([128, FT], f32, name=f"a1_{k}")
                nc.scalar.dma_start(a1k, a1_d[k * 128 : (k + 1) * 128, :])
                a1_t.append(a1k)
            gt_t = []
            for kc in range(2):
                gtk = cp.tile([128, rows], f32, name=f"gt_{kc}")
                nc.scalar.dma_start(gtk, gtt_d[kc * 128 : (kc + 1) * 128, :])
                gt_t.append(gtk)
            io_t = cp.tile([N_IO, rows], f32, name="io_t")
            nc.scalar.dma_start(io_t, iot_d[:, :])
            w1io_t = cp.tile([N_IO, 256], f32, name="w1io_t")
            nc.scalar.dma_start(w1io_t, w1io_d[:, :])
            cw2_t = cp.tile([128, 256], f32, name="cw2_t")
            # cw2 is [256(K), 128(M)]; lhsT k-chunks side by side in free dim
            nc.scalar.dma_start(cw2_t[:, 0:128], cw2_d[0:128, :])
            nc.scalar.dma_start(cw2_t[:, 128:256], cw2_d[128:256, :])
            cw3_t = cp.tile([128, 1], f32, name="cw3_t")
            nc.scalar.dma_start(cw3_t, cw3_d[:, :])
            cb2_t = cp.tile([128, 1], f32, name="cb2_t")
            nc.scalar.dma_start(cb2_t, cb2_d[:].rearrange("p -> p ()"))
            w2h_t = cp.tile([128, 3], f32, name="w2h_t")
            nc.scalar.dma_start(w2h_t, w2h_d[:, :])
            scal_t = cp.tile([1, 8], f32, name="scal_t")
            nc.scalar.dma_start(scal_t, scal_d[:].rearrange("s -> () s"))
            ident_t = cp.tile([128, 128], f32, name="ident_t")
            nc.scalar.dma_start(ident_t, ident_d[:, :])
            b1_t = []
            for m in range(5):
                b1m = cp.tile([128, 1], f32, name=f"b1_{m}")
                nc.scalar.dma_start(
                    b1m, b1_d[m * 128 : (m + 1) * 128].rearrange("p -> p ()")
                )
                b1_t.append(b1m)
            ones4 = cp.tile([4, 1], f32, name="ones4")
            nc.vector.memset(ones4, 1.0)

            # ---- connections reduce (DMA-bound bulk, sync-engine ring) ----
            ncT = cp.tile([1, rows], f32, name="ncT")
            for j in range(rc):
                plan = CONN_PLAN_LAST if j == rc - 1 else CONN_PLAN
                pcol = cp.tile([128, len(plan)], f32, name=f"pcol_{j}")
                off = 0
                for i, (w, eng) in enumerate(plan):
                    ct = connp.tile([128, 8192], f32, name="ct", tag="ct")
                    cta = ct[:, :w]
                    nc.sync.dma_start(
                        cta, conn_d[j * 128 : (j + 1) * 128, off : off + w]
                    )
                    off += w
                    if eng == "D":
                        nc.vector.tensor_scalar(
                            cta, cta, 0.0, None, ALU.add, ALU.add,
                            accum_out=pcol[:, i : i + 1],
                        )
                    else:
                        nc.scalar.activation(
                            cta, cta, AF.Copy, accum_out=pcol[:, i : i + 1]
                        )
                ncol = cp.tile([128, 1], f32, name=f"ncol_{j}")
                nc.vector.reduce_sum(ncol, pcol, axis=AX.X)
                # flip row-major [128, 1] -> free-major [1, 128] on the PE
                ptr = pvp.tile([1, 128], f32, name=f"ptr_{j}", tag="vp")
                nc.tensor.transpose(ptr, ncol, ident_t)
                nc.vector.tensor_copy(ncT[:, j * 128 : (j + 1) * 128], ptr)

            # ---- one-hot of gate types, transposed layout [1024, rows] ----
            oh = []
            for t in range(N_TYPES):
                for kc in range(2):
                    ohk = cp.tile([128, rows], f32, name=f"oh_{t}_{kc}")
                    nc.vector.tensor_scalar(ohk, gt_t[kc], float(t), None, ALU.is_equal)
                    oh.append(ohk)

            # ---- first layer: h1_T[f, r] = sum_k A1[k, f] * onehot[k, r] ----
            h1_sb = []
            for m in range(5):
                ph = php.tile([128, rows], f32, name="h1p", tag="h1p")
                for k in range(8):
                    last = (k == 7) and m not in (3, 4)
                    nc.tensor.matmul(
                        ph, a1_t[k][:, m * 128 : (m + 1) * 128], oh[k],
                        start=(k == 0), stop=last,
                    )
                if m in (3, 4):
                    nc.tensor.matmul(
                        ph, w1io_t[:, (m - 3) * 128 : (m - 2) * 128], io_t,
                        start=False, stop=True,
                    )
                h1m = cp.tile([128, rows], f32, name=f"h1_{m}")
                nc.scalar.activation(h1m, ph, AF.Relu, bias=b1_t[m])
                h1_sb.append(h1m)

            # counts chunk: rows 640:644 of A1 are per-type indicator columns
            pcnt = pvp.tile([4, rows], f32, name="pcnt", tag="vp")
            for k in range(8):
                nc.tensor.matmul(
                    pcnt, a1_t[k][:, F1 : F1 + 4], oh[k],
                    start=(k == 0), stop=(k == 7),
                )

            # ---- gate-type entropy pieces (feature-major [4, rows]) ----
            probs = vtile("probs", 4)
            nc.scalar.activation(probs, pcnt, AF.Copy, scale=1.0 / G)
            pmax = vtile("pmax", 4)
            nc.vector.tensor_scalar_max(pmax, probs, 1e-30)
            lnp = vtile("lnp", 4)
            nc.scalar.activation(lnp, pmax, AF.Ln)
            plp = vtile("plp", 4)
            nc.vector.tensor_tensor(plp, probs, lnp, ALU.mult)
            pge = pvp.tile([1, rows], f32, name="pge", tag="vp")
            nc.tensor.matmul(pge, ones4, plp, start=True, stop=True)
            ge_sb = cp.tile([1, rows], f32, name="ge_sb")
            nc.vector.tensor_copy(ge_sb, pge)

            # ---- heads ----
            def softplus(x, tag):
                ax = vtile(f"ax_{tag}")
                nc.scalar.activation(ax, x, AF.Abs)
                ex = vtile(f"ex_{tag}")
                nc.scalar.activation(ex, ax, AF.Exp, scale=-1.0)
                ll = vtile(f"ll_{tag}")
                nc.scalar.activation(ll, ex, AF.Ln, bias=1.0)
                mx = vtile(f"mx_{tag}")
                nc.vector.tensor_scalar_max(mx, x, 0.0)
                return ll, mx

            # power head (m=0): softplus(h1 @ pw2 + pb2), + conn term later
            pp = pvp.tile([1, rows], f32, name="pp", tag="vp")
            nc.tensor.matmul(pp, w2h_t[:, 0:1], h1_sb[0], start=True, stop=True)
            xp = vtile("xp")
            nc.scalar.activation(xp, pp, AF.Identity, bias=scal_t[:, 0:1])
            ll_p, mx_p = softplus(xp, "p")
            sp_p = cp.tile([1, rows], f32, name="sp_p")
            nc.vector.tensor_tensor(sp_p, mx_p, ll_p, ALU.add)

            # stability head (m=1): sigmoid(h1 @ nw2 + nb2) * exp(-1)
            pn = pvp.tile([1, rows], f32, name="pn", tag="vp")
            nc.tensor.matmul(pn, w2h_t[:, 1:2], h1_sb[1], start=True, stop=True)
            sg = vtile("sg")
            nc.scalar.activation(sg, pn, AF.Sigmoid, bias=scal_t[:, 1:2])
            stab = vtile("stab")
            nc.vector.tensor_scalar_mul(stab, sg, math.exp(-1.0))
            nc.scalar.dma_start(outs_d["stability"][:].rearrange("r -> () r"), stab)

            # delay head (m=2): softplus(h1 @ dw2 + db2)
            pd = pvp.tile([1, rows], f32, name="pd", tag="vp")
            nc.tensor.matmul(pd, w2h_t[:, 2:3], h1_sb[2], start=True, stop=True)
            xd = vtile("xd")
            nc.scalar.activation(xd, pd, AF.Identity, bias=scal_t[:, 2:3])
            ll_d, mx_d = softplus(xd, "d")
            spd = vtile("spd")
            nc.vector.tensor_tensor(spd, mx_d, ll_d, ALU.add)
            nc.scalar.dma_start(outs_d["delay"][:].rearrange("r -> () r"), spd)

            # correctness head (m=3,4): 3-layer MLP
            ph2 = php.tile([128, rows], f32, name="h2p", tag="h1p")
            nc.tensor.matmul(ph2, cw2_t[:, 0:128], h1_sb[3], start=True, stop=False)
            nc.tensor.matmul(ph2, cw2_t[:, 128:256], h1_sb[4], start=False, stop=True)
            h2 = cp.tile([128, rows], f32, name="h2")
            nc.scalar.activation(h2, ph2, AF.Relu, bias=cb2_t)
            pcr = pvp.tile([1, rows], f32, name="pcr", tag="vp")
            nc.tensor.matmul(pcr, cw3_t, h2, start=True, stop=True)
            corr = vtile("corr")
            nc.scalar.activation(corr, pcr, AF.Sigmoid, bias=scal_t[:, 3:4])
            nc.scalar.dma_start(outs_d["correctness"][:].rearrange("r -> () r"), corr)

            # ---- energy = softplus_power + 0.5 * num_conn * 0.1 ----
            e05 = vtile("e05")
            nc.vector.tensor_scalar_mul(e05, ncT, 0.05)
            energy = vtile("energy_v")
            nc.vector.tensor_tensor(energy, sp_p, e05, ALU.add)
            nc.scalar.dma_start(outs_d["energy"][:].rearrange("r -> () r"), energy)

            # ---- entropy = gate_ent + binary entropy of density ----
            dens = vtile("dens")
            nc.vector.tensor_scalar_mul(dens, ncT, 1.0 / CONN_F)
            dcl = vtile("dcl")
            nc.vector.tensor_scalar(dcl, dens, 1e-12, 1.0 - 1e-12, ALU.max, ALU.min)
            ln_d = vtile("ln_d")
            nc.scalar.activation(ln_d, dcl, AF.Ln)
            om = vtile("om")
            nc.vector.tensor_scalar(om, dcl, -1.0, 1.0, ALU.mult, ALU.add)
            ln_o = vtile("ln_o")
            nc.scalar.activation(ln_o, om, AF.Ln)
            t1 = vtile("t1")
            nc.vector.tensor_tensor(t1, dcl, ln_d, ALU.mult)
            t2 = vtile("t2")
            nc.vector.tensor_tensor(t2, om, ln_o, ALU.mult)
            s1 = vtile("s1")
            nc.vector.tensor_tensor(s1, t1, t2, ALU.add)
            s2 = vtile("s2")
            nc.vector.tensor_tensor(s2, s1, ge_sb, ALU.add)
            ent = vtile("ent")
            nc.vector.tensor_scalar_mul(ent, s2, -LN2_INV)
            nc.scalar.dma_start(outs_d["entropy"][:].rearrange("r -> () r"), ent)

    nc.compile()
    return nc


_NC_CACHE = {}


def _get_nc(rows=R):
    if rows not in _NC_CACHE:
        _NC_CACHE[rows] = build_program(rows)
    return _NC_CACHE[rows]


def host_prep(inputs):
    """Transform full inputs into the device tensors (shared + per-core)."""
    gt = np.asarray(inputs["gate_types"])
    conn = np.asarray(inputs["connections"], dtype=np.float32).reshape(B, CONN_F)
    xin = np.asarray(inputs["inputs"], dtype=np.float32)
    xout = np.asarray(inputs["outputs"], dtype=np.float32)
    emb = np.asarray(inputs["emb"], dtype=np.float32)
    pw1, pb1 = np.asarray(inputs["pw1"]), np.asarray(inputs["pb1"])
    pw2, pb2 = np.asarray(inputs["pw2"]), np.asarray(inputs["pb2"])
    dw1, db1 = np.asarray(inputs["dw1"]), np.asarray(inputs["db1"])
    dw2, db2 = np.asarray(inputs["dw2"]), np.asarray(inputs["db2"])
    nw1, nb1 = np.asarray(inputs["nw1"]), np.asarray(inputs["nb1"])
    nw2, nb2 = np.asarray(inputs["nw2"]), np.asarray(inputs["nb2"])
    cw1, cb1 = np.asarray(inputs["cw1"]), np.asarray(inputs["cb1"])
    cw2, cb2 = np.asarray(inputs["cw2"]), np.asarray(inputs["cb2"])
    cw3, cb3 = np.asarray(inputs["cw3"]), np.asarray(inputs["cb3"])

    w1 = np.concatenate([pw1, nw1, dw1, cw1[:CE]], axis=1)  # [8192, 640]
    a1 = np.einsum(
        "td,gdf->tgf",
        emb.astype(np.float64),
        w1.reshape(G, D, F1).astype(np.float64),
    ).reshape(K1, F1)
    cnt_cols = np.zeros((N_TYPES, G, N_TYPES), np.float64)
    for t in range(N_TYPES):
        cnt_cols[t, :, t] = 1.0
    a1e = np.concatenate([a1, cnt_cols.reshape(K1, N_TYPES)], axis=1).astype(np.float32)

    shared = {
        "a1": a1e,
        "b1": np.concatenate([pb1, nb1, db1, cb1]).astype(np.float32),
        "w1io": np.ascontiguousarray(cw1[CE:]).astype(np.float32),
        "cw2": np.ascontiguousarray(cw2).astype(np.float32),
        "cw3": np.ascontiguousarray(cw3).astype(np.float32),
        "cb2": np.ascontiguousarray(cb2).astype(np.float32),
        "w2h": np.stack([pw2[:, 0], nw2[:, 0], dw2[:, 0]], axis=1).astype(np.float32),
        "scal": np.array(
            [pb2[0], nb2[0], db2[0], cb3[0], 0, 0, 0, 0], np.float32
        ),
        "ident": np.eye(128, dtype=np.float32),
    }
    gtt = np.ascontiguousarray(gt.T).astype(np.float32)  # [256, 4096]
    iot = np.ascontiguousarray(np.concatenate([xin, xout], axis=1).T)  # [12, 4096]
    return conn, gtt, iot, shared


def make_in_maps(inputs, n_cores=N_CORES, rows=R):
    conn, gtt, iot, shared = host_prep(inputs)
    in_maps = []
    for c in range(n_cores):
        sl = slice(c * rows, (c + 1) * rows)
        m = dict(shared)
        m["conn"] = np.ascontiguousarray(conn[sl])
        m["gtt"] = np.ascontiguousarray(gtt[:, sl])
        m["iot"] = np.ascontiguousarray(iot[:, sl])
        in_maps.append(m)
    return in_maps


def kernel(**inputs):
    nc = _get_nc(R)
    in_maps = make_in_maps(inputs)
    res = run_bass_kernel_spmd(nc, in_maps, core_ids=list(range(N_CORES)))
    outs = res.results
    names = ["energy", "entropy", "stability", "correctness", "delay"]
    return tuple(
        np.concatenate([np.asarray(outs[c][n]) for c in range(N_CORES)]) for n in names
    )

